# revision 28
# baseline (speedup 1.0000x reference)
"""Trainium2 Bass kernel for nn_COLoss_45457933860953 (raw-bass version).

Loss = mean over all pixels of weighted -log(conf gathered by instance)
     + mean over batches of (masked offset MSE sum / fg count).

Data-parallel over the batch dim: 16 batches -> 8 cores x 2 batches.

Host-side compression (loss tolerance 2e-2, measured quantization error
~2e-4): conf channels as fp8-e4m3 (they only feed the gather + Ln; ACT
is dtype-independent), offsets/gt as bf16 (keeps DVE in packed 2x perf
mode), instance mask as bf16 0/1 (arithmetic mask; its high byte is the
int8 predicate copy_predicated needs). Everything for one col-chunk is
one contiguous blob -> one fully-contiguous 2D DMA per chunk:

  per partition, ck cols: [ mask bf16 | c0 fp8 | c1 fp8 | o0 | o1 | g0 | g1 ]

Partition p holds image rows 4p..4p+3 flattened (2048 cols per batch),
col-chunks (1024, 1024) per batch.

RAW bass (no TileContext): the Tile framework costs ~7us of pool-alloc
barriers before the first DMA trigger and ~6us of per-semaphore
postamble teardown (~57 serial waits per engine).  With a hand-built
static schedule we use 10 semaphores total, and a post-pass hoists the
four input-DMA triggers to the head of the preamble block so descriptor
generation starts as soon as the SP queue is live (~6us, vs ~7.3us
after the all-engine barrier).  All four transfers are issued
back-to-back (the rings process them near-FIFO at ~430 B/ns; never put
waits between them -- a blocked SP queue in the preamble holds the
all-engine barrier hostage).

Per chunk s, engines:
  PE  : fg count += ones^T @ mask strips -> psum per batch
        S2 += ones^T @ (m*log g) strips  -> psum (one accumulator)
  DVE : gather g=conf[inst] (copy_predicated on fp8 bytes),
        d_c = gt_c - off_c, d_c *= m, lgm = log(g) * m
  ACT : log g -> bf16 (accum S1 -> psum), Square(d_c*m) (accum -> psum)

Outputs: res[128,12] (S1 per chunk + off sq-sums per chunk*ch),
cnt[1,1536] (= count_b0 | count_b1 | S2 columns). Host combines in
float64.
"""

import sys

if "/opt/trn_rl_repo" not in sys.path:
    sys.path.insert(0, "/opt/trn_rl_repo")

import ml_dtypes
import numpy as np

import concourse.bass as bass
from concourse import mybir
from concourse.bass_utils import run_bass_kernel_spmd

B, C, H, W = 16, 2, 512, 512
NCORES = 8
BPC = B // NCORES            # batches per core
P = 128                      # SBUF partitions
FREE = (H * W) // P          # 2048 cols per partition per image
CHUNKS = (1024, 1024)        # equal col-chunks per batch
NCHUNK = len(CHUNKS)
NSETS = BPC * NCHUNK         # chunk-sets per core (4)
BPE = 12                     # blob bytes per col (2+1+1+2+2+2+2)

BF16 = mybir.dt.bfloat16
FP8 = mybir.dt.float8e4
F32 = mybir.dt.float32
AF = mybir.ActivationFunctionType
ALU = mybir.AluOpType
NPBF16 = ml_dtypes.bfloat16
NPFP8 = ml_dtypes.float8_e4m3fn


def _legalize_single_wait(nc):
    """Drop the EVENT_SEMAPHORE_RANGE_CLEAR InstISA this toolchain's walrus
    rejects, and hoist surplus sync-waits (>1) onto standalone single-wait
    carriers on the same queue (prefix waits on an in-order queue are
    semantically identical to instruction waits)."""
    cnt = 0
    for f in nc.m.functions:
        for blk in f.blocks:
            out = []
            for ins in blk.instructions:
                nm = type(ins).__name__
                if (nm == "InstISA" and
                        getattr(ins, "op_name", None) ==
                        "EVENT_SEMAPHORE_RANGE_CLEAR"):
                    continue
                si = getattr(ins, "sync_info", None)
                if si is not None and si.on_wait and len(si.on_wait) > 1:
                    waits = list(si.on_wait)
                    for w in waits[:-1]:
                        cnt += 1
                        out.append(mybir.InstEventSemaphore(
                            name=f"{ins.name}-hoist{cnt}",
                            engine=ins.engine,
                            ins=[], outs=[],
                            sync_info=mybir.SyncInfo(on_wait=[w],
                                                     on_update=[]),
                        ))
                    ins.sync_info = mybir.SyncInfo(
                        on_wait=[waits[-1]], on_update=list(si.on_update))
                out.append(ins)
            blk.instructions = out
    return nc


def _hoist_input_dmas(nc):
    """Move the SP-engine input-DMA prefix (4 DMACopy triggers + their
    serialize waits) from the main block to the FRONT of the preamble
    block.  The preamble's all-engine barrier otherwise delays the first
    descriptor generation to ~7.3us; hoisted, the SP queue generates
    descriptors as soon as it is live (~4.6us) while the other engines
    run their preamble.  Only the SP-engine subsequence order matters,
    and the data dependencies (dsem waits by compute) are unchanged."""
    blocks = nc.m.functions[0].blocks
    pre = blocks[0]
    sp = mybir.EngineType.SP
    main = next(b for b in blocks[1:]
                if any(type(i).__name__ == "InstDMACopy" and i.engine == sp
                       for i in b.instructions))
    prefix, rest, moved_dma = [], [], 0
    for ins in main.instructions:
        if moved_dma < NSETS and ins.engine == sp and type(ins).__name__ in (
                "InstDMACopy", "InstEventSemaphore"):
            prefix.append(ins)
            if type(ins).__name__ == "InstDMACopy":
                moved_dma += 1
        else:
            rest.append(ins)
    assert moved_dma == NSETS
    main.instructions = rest
    pre.instructions = prefix + pre.instructions
    return nc


def build_nc(legalize=True):
    nc = bass.Bass("TRN2", target_bir_lowering=False, debug=False,
                   num_devices=NCORES)
    blob = nc.dram_tensor("blob", [NSETS, P, BPE * CHUNKS[0] // 2],
                          BF16, kind="ExternalInput")
    res_d = nc.dram_tensor("res", [P, 3 * NSETS], F32, kind="ExternalOutput")
    cnt_d = nc.dram_tensor("cnt", [1, 3 * 512], F32, kind="ExternalOutput")

    ck0, ck1 = CHUNKS
    from contextlib import ExitStack
    with ExitStack() as es:
        block = es.enter_context(nc.Block())
        dsems = [es.enter_context(nc.semaphore(f"ds{i}")) for i in range(4)]
        vs = es.enter_context(nc.semaphore("vs"))    # DVE progress
        as_ = es.enter_context(nc.semaphore("as_"))  # ACT progress
        ps = es.enter_context(nc.semaphore("ps"))    # PE progress
        os_ = es.enter_context(nc.semaphore("os_"))  # output DMA done
        cks = [ck0, ck1, ck0, ck1]
        tiles = [es.enter_context(
            nc.sbuf_tensor(f"t{i}", [P, BPE * cks[i] // 2], BF16))
            for i in range(4)]
        scratch = [tuple(es.enter_context(
            nc.sbuf_tensor(f"{nm}{i}", [P, cks[i]], BF16))
            for nm in ("d0", "d1", "lg")) for i in range(4)]
        ones = es.enter_context(nc.sbuf_tensor("ones", [P, 1], BF16))
        dummy = es.enter_context(nc.sbuf_tensor("dmy", [P, 1], BF16))
        res_sb = es.enter_context(
            nc.sbuf_tensor("res_sb", [P, 3 * NSETS], F32))
        cnt_sb = es.enter_context(
            nc.sbuf_tensor("cnt_sb", [1, 3 * 512], F32))
        racc = es.enter_context(
            nc.psum_tensor("racc", [P, 3 * NSETS], F32))
        cnt_ps = [es.enter_context(nc.psum_tensor(f"cntp{b}", [1, 512], F32))
                  for b in range(BPC)]
        s2p = es.enter_context(nc.psum_tensor("s2p", [1, 512], F32))

        def fields(s):
            T, ck = tiles[s], cks[s]
            m = T[:, 0:ck]
            conf8 = T[:, ck:2 * ck].bitcast(FP8)
            m_i8 = m.bitcast(mybir.dt.int8).rearrange(
                "p (k two) -> p k two", two=2)[:, :, 1]
            return dict(m=m, m_i8=m_i8, c0=conf8[:, 0:ck],
                        c1=conf8[:, ck:2 * ck],
                        o0=T[:, 2 * ck:3 * ck], o1=T[:, 3 * ck:4 * ck],
                        g0=T[:, 4 * ck:5 * ck], g1=T[:, 5 * ck:6 * ck])

        # semaphore value bookkeeping (emission-order counters)
        V = {"vs": 0, "as": 0, "ps": 0}
        # per-chunk landmark values, filled as streams are emitted
        vget = {}   # ("gather"|"mul0"|"mul1"|"lgm", s) -> vs value
        aget = {}   # ("ln", s) -> as_ value
        pget = {}   # ("cnt", b) / ("s2",) -> ps value

        # ---------------- DVE ----------------
        @block.vector
        def _(vector):
            v = nc.vector
            v.memset(ones[:], 1.0).then_inc(vs, 1)
            V["vs"] += 1
            for s in range(NSETS):
                f = fields(s)
                d0, d1, lg = scratch[s]
                vector.wait_ge(dsems[s], 16)
                v.copy_predicated(f["c0"], f["m_i8"], f["c1"]) \
                    .then_inc(vs, 1)
                V["vs"] += 1
                vget[("gather", s)] = V["vs"]
                v.tensor_sub(d0[:], f["g0"], f["o0"])
                v.tensor_sub(d1[:], f["g1"], f["o1"]).then_inc(vs, 1)
                V["vs"] += 1
                # same-queue RAW (sub -> mul) needs an explicit token for
                # the race model; in-order completion makes one wait enough
                vector.wait_ge(vs, V["vs"])
                v.tensor_mul(d0[:], d0[:], f["m"]).then_inc(vs, 1)
                V["vs"] += 1
                vget[("mul0", s)] = V["vs"]
                v.tensor_mul(d1[:], d1[:], f["m"]).then_inc(vs, 1)
                V["vs"] += 1
                vget[("mul1", s)] = V["vs"]
                # lgm = log(g) * m for the PE S2 reduction
                vector.wait_ge(as_, s + 1)         # Ln of chunk s done
                v.tensor_mul(lg[:], lg[:], f["m"]).then_inc(vs, 1)
                V["vs"] += 1
                vget[("lgm", s)] = V["vs"]
                if s == 1 or s == 3:
                    b = s // 2
                    vector.wait_ge(ps, b + 1)   # cnt group b stopped
                    v.tensor_copy(cnt_sb[0:1, 512 * b:512 * (b + 1)],
                                  cnt_ps[b][:]).then_inc(vs, 1)
                    V["vs"] += 1
                    vget[("cntcp", b)] = V["vs"]
            # S2 psum -> sbuf (ps==3 once the S2 group stops)
            vector.wait_ge(ps, 3)
            v.tensor_copy(cnt_sb[0:1, 1024:1536], s2p[:]).then_inc(vs, 1)
            V["vs"] += 1
            vget[("s2cp",)] = V["vs"]
            # all ACT accums done -> drain racc psum to sbuf
            vector.wait_ge(as_, NSETS + 1)
            v.tensor_copy(res_sb[:], racc[:]).then_inc(vs, 1)
            V["vs"] += 1
            vget[("rescp",)] = V["vs"]

        # ---------------- ACT ----------------
        @block.scalar
        def _(scalar):
            sc = nc.scalar
            scalar.wait_ge(vs, 1)              # ones ready
            sc.activation(dummy[:], ones[:], AF.Ln)   # table prefetch
            for s in range(NSETS):
                f = fields(s)
                d0, d1, lg = scratch[s]
                scalar.wait_ge(vs, vget[("gather", s)])
                sc.activation(lg[:], f["c0"], AF.Ln,
                              accum_out=racc[:, s:s + 1]).then_inc(as_, 1)
                V["as"] += 1
                aget[("ln", s)] = V["as"]
                scalar.wait_ge(vs, vget[("mul0", s)])
                i = sc.activation(d0[:], d0[:], AF.Square,
                                  accum_out=racc[:, NSETS + 2 * s:
                                                 NSETS + 2 * s + 1])
                scalar.wait_ge(vs, vget[("mul1", s)])
                i = sc.activation(d1[:], d1[:], AF.Square,
                                  accum_out=racc[:, NSETS + 2 * s + 1:
                                                 NSETS + 2 * s + 2])
                if s == NSETS - 1:
                    i.then_inc(as_, 1)         # final accumulate landmark
                    V["as"] += 1

        # ---------------- PE -----------------
        @block.tensor
        def _(tensor):
            t = nc.tensor
            # per-batch count groups + one global S2 group
            cnt_started = [False, False]
            s2_started = False
            tensor.wait_ge(vs, 1)              # ones ready
            for s in range(NSETS):
                f = fields(s)
                ck = cks[s]
                b = s // 2
                d0, d1, lg = scratch[s]
                tensor.wait_ge(dsems[s], 16)
                off = 0
                while off < ck:
                    w = min(512, ck - off)
                    last = (s % 2 == 1) and (off + w >= ck)
                    i = t.matmul(cnt_ps[b][:, 0:w], ones[:],
                                 f["m"][:, off:off + w],
                                 start=not cnt_started[b], stop=last,
                                 skip_group_check=True)
                    cnt_started[b] = True
                    off += w
                    if last:
                        i.then_inc(ps, 1)
                        V["ps"] += 1
                        pget[("cnt", b)] = V["ps"]
                tensor.wait_ge(vs, vget[("lgm", s)])
                off = 0
                while off < ck:
                    w = min(512, ck - off)
                    last = (s == NSETS - 1) and (off + w >= ck)
                    i = t.matmul(s2p[:, 0:w], ones[:], lg[:, off:off + w],
                                 start=not s2_started, stop=last,
                                 skip_group_check=True)
                    s2_started = True
                    off += w
                    if last:
                        i.then_inc(ps, 1)
                        V["ps"] += 1
                        pget[("s2",)] = V["ps"]

        # fix the DVE waits that referenced PE progress: recompute final
        # values now that PE stream is emitted (ps: cnt_b0=1 after chunk1,
        # cnt_b1=2 after chunk3, s2=3 at end) -- the wait_ge calls above
        # used these exact constants; assert they match.
        assert pget[("cnt", 0)] == 1 and pget[("cnt", 1)] == 2
        assert pget[("s2",)] == 3 and V["as"] == NSETS + 1

        # ---------------- SP (DMAs) ----------------
        @block.sync
        def _(sync):
            # all four input transfers issued back-to-back; the rings
            # process their descriptors mostly FIFO at ~430 B/ns, and
            # _hoist_input_dmas moves these triggers to the head of the
            # preamble so generation starts as soon as SP is live (~6us).
            # NOTE: never put waits between them -- a blocked SP queue in
            # the preamble holds the all-engine barrier hostage.
            for s in range(NSETS):
                nc.sync.dma_start(tiles[s][:], blob[s]).then_inc(dsems[s], 16)
            sync.wait_ge(vs, vget[("rescp",)])
            nc.sync.dma_start(res_d[:, :], res_sb[:]).then_inc(os_, 16)
            sync.wait_ge(vs, vget[("s2cp",)])
            nc.sync.dma_start(cnt_d[:, :], cnt_sb[:]).then_inc(os_, 16)
            sync.wait_ge(os_, 32)

    if not legalize:
        return nc
    return _legalize_single_wait(_hoist_input_dmas(nc))


_NC = None


def _get_nc():
    global _NC
    if _NC is None:
        _NC = build_nc()
    return _NC


def make_in_maps(confidence, offset, instance, gt_offset):
    conf = np.ascontiguousarray(confidence, dtype=np.float32) \
        .reshape(B, C, P, FREE).astype(NPFP8)
    off = np.ascontiguousarray(offset, dtype=np.float32) \
        .reshape(B, 2, P, FREE).astype(NPBF16)
    gto = np.ascontiguousarray(gt_offset, dtype=np.float32) \
        .reshape(B, 2, P, FREE).astype(NPBF16)
    mask = (np.asarray(instance).reshape(B, P, FREE) != 0).astype(NPBF16)

    def pack(b, lo, hi):
        # byte-level pack: [mask bf16 | c0 fp8 | c1 fp8 | o0 o1 g0 g1 bf16]
        parts = [np.ascontiguousarray(mask[b][:, lo:hi]).view(np.uint8),
                 np.ascontiguousarray(conf[b, 0][:, lo:hi]).view(np.uint8),
                 np.ascontiguousarray(conf[b, 1][:, lo:hi]).view(np.uint8),
                 np.ascontiguousarray(off[b, 0][:, lo:hi]).view(np.uint8),
                 np.ascontiguousarray(off[b, 1][:, lo:hi]).view(np.uint8),
                 np.ascontiguousarray(gto[b, 0][:, lo:hi]).view(np.uint8),
                 np.ascontiguousarray(gto[b, 1][:, lo:hi]).view(np.uint8)]
        return np.concatenate(parts, axis=1).view(NPBF16)

    edges = np.cumsum((0,) + CHUNKS)
    in_maps = []
    for k in range(NCORES):
        bs = [BPC * k + i for i in range(BPC)]
        blobs = np.stack([pack(b, edges[c], edges[c + 1])
                          for b in bs for c in range(NCHUNK)])
        in_maps.append({"blob": blobs})
    return in_maps


def combine_partials(parts):
    """parts: list of 8 dicts (res [P,12], cnt [1,1536]) -> loss."""
    s1 = sum(p["res"][:, 0:NSETS].sum(dtype=np.float64) for p in parts)
    s2 = sum(p["cnt"][0, 1024:1536].sum(dtype=np.float64) for p in parts)
    n = float(B * H * W)
    conf_loss = -(0.4 * s1 + 0.6 * s2) / n
    off_loss = 0.0
    for p in parts:
        for bi in range(BPC):
            lo = NSETS + 2 * NCHUNK * bi
            s = p["res"][:, lo:lo + 2 * NCHUNK].sum(dtype=np.float64)
            cntb = p["cnt"][0, 512 * bi:512 * (bi + 1)].sum(dtype=np.float64)
            if cntb > 0.5:
                off_loss += s / cntb
    off_loss /= B
    return conf_loss + off_loss


def kernel(confidence, offset, instance, gt_offset):
    nc = _get_nc()
    in_maps = make_in_maps(confidence, offset, instance, gt_offset)
    res = run_bass_kernel_spmd(nc, in_maps, core_ids=list(range(NCORES)))
    parts = [{k: np.asarray(r[k], dtype=np.float64)
              for k in ("res", "cnt")} for r in res.results]
    return np.array(combine_partials(parts), dtype=np.float32)


# revision 29
# speedup vs baseline: 1.0611x; 1.0611x over previous
"""Trainium2 Bass kernel for nn_COLoss_45457933860953 (raw-bass version).

Loss = mean over all pixels of weighted -log(conf gathered by instance)
     + mean over batches of (masked offset MSE sum / fg count).

Data-parallel over the batch dim: 16 batches -> 8 cores x 2 batches.

Host-side compression (loss tolerance 2e-2, measured quantization error
~2e-4): conf channels as fp8-e4m3 (they only feed the gather + Ln; ACT
is dtype-independent), offsets/gt as bf16 (keeps DVE in packed 2x perf
mode), instance mask as bf16 0/1 (arithmetic mask; its high byte is the
int8 predicate copy_predicated needs). Everything for one col-chunk is
one contiguous blob -> one fully-contiguous 2D DMA per chunk:

  per partition, ck cols: [ mask bf16 | c0 fp8 | c1 fp8 | o0 | o1 | g0 | g1 ]

Partition p holds image rows 4p..4p+3 flattened (2048 cols per batch),
col-chunks (1024, 1024) per batch.

RAW bass (no TileContext): the Tile framework costs ~7us of pool-alloc
barriers before the first DMA trigger and ~6us of per-semaphore
postamble teardown (~57 serial waits per engine).  With a hand-built
static schedule we use 10 semaphores total, and a post-pass hoists the
four input-DMA triggers to the head of the preamble block so descriptor
generation starts as soon as the SP queue is live (~6us, vs ~7.3us
after the all-engine barrier).  All four transfers are issued
back-to-back (the rings process them near-FIFO at ~430 B/ns; never put
waits between them -- a blocked SP queue in the preamble holds the
all-engine barrier hostage).

Per chunk s, engines:
  PE  : fg count += ones^T @ mask strips -> psum per batch
        S2 += ones^T @ (m*log g) strips  -> psum (one accumulator)
  DVE : gather g=conf[inst] (copy_predicated on fp8 bytes),
        d_c = gt_c - off_c, d_c *= m, lgm = log(g) * m
  ACT : log g -> bf16 (accum S1 -> psum), Square(d_c*m) (accum -> psum)

Outputs: res[128,12] (S1 per chunk + off sq-sums per chunk*ch),
cnt[1,1536] (= count_b0 | count_b1 | S2 columns). Host combines in
float64.
"""

import sys

if "/opt/trn_rl_repo" not in sys.path:
    sys.path.insert(0, "/opt/trn_rl_repo")

import ml_dtypes
import numpy as np

import concourse.bass as bass
from concourse import mybir
from concourse.bass_utils import run_bass_kernel_spmd

B, C, H, W = 16, 2, 512, 512
NCORES = 8
BPC = B // NCORES            # batches per core
P = 128                      # SBUF partitions
FREE = (H * W) // P          # 2048 cols per partition per image
# chunk tables: batch 0 leads with two 512-col chunks so compute starts
# ~3us earlier (smaller first transfer); the rest are 1024-col chunks.
CKS = (512, 512, 1024, 1024, 1024)       # cols per chunk-set
COLR = ((0, 512), (512, 1024), (1024, 2048), (0, 1024), (1024, 2048))
SBATCH = (0, 0, 0, 1, 1)                 # batch of each chunk-set
BLAST = (False, False, True, False, True)  # last chunk of its batch
NSETS = len(CKS)
BPE = 12                     # blob bytes per col (2+1+1+2+2+2+2)

BF16 = mybir.dt.bfloat16
FP8 = mybir.dt.float8e4
F32 = mybir.dt.float32
AF = mybir.ActivationFunctionType
ALU = mybir.AluOpType
NPBF16 = ml_dtypes.bfloat16
NPFP8 = ml_dtypes.float8_e4m3fn


def _legalize_single_wait(nc):
    """Drop the EVENT_SEMAPHORE_RANGE_CLEAR InstISA this toolchain's walrus
    rejects, and hoist surplus sync-waits (>1) onto standalone single-wait
    carriers on the same queue (prefix waits on an in-order queue are
    semantically identical to instruction waits)."""
    cnt = 0
    for f in nc.m.functions:
        for blk in f.blocks:
            out = []
            for ins in blk.instructions:
                nm = type(ins).__name__
                if (nm == "InstISA" and
                        getattr(ins, "op_name", None) ==
                        "EVENT_SEMAPHORE_RANGE_CLEAR"):
                    continue
                si = getattr(ins, "sync_info", None)
                if si is not None and si.on_wait and len(si.on_wait) > 1:
                    waits = list(si.on_wait)
                    for w in waits[:-1]:
                        cnt += 1
                        out.append(mybir.InstEventSemaphore(
                            name=f"{ins.name}-hoist{cnt}",
                            engine=ins.engine,
                            ins=[], outs=[],
                            sync_info=mybir.SyncInfo(on_wait=[w],
                                                     on_update=[]),
                        ))
                    ins.sync_info = mybir.SyncInfo(
                        on_wait=[waits[-1]], on_update=list(si.on_update))
                out.append(ins)
            blk.instructions = out
    return nc


def _hoist_input_dmas(nc):
    """Move the SP-engine input-DMA prefix (4 DMACopy triggers + their
    serialize waits) from the main block to the FRONT of the preamble
    block.  The preamble's all-engine barrier otherwise delays the first
    descriptor generation to ~7.3us; hoisted, the SP queue generates
    descriptors as soon as it is live (~4.6us) while the other engines
    run their preamble.  Only the SP-engine subsequence order matters,
    and the data dependencies (dsem waits by compute) are unchanged."""
    blocks = nc.m.functions[0].blocks
    pre = blocks[0]
    sp = mybir.EngineType.SP
    main = next(b for b in blocks[1:]
                if any(type(i).__name__ == "InstDMACopy" and i.engine == sp
                       for i in b.instructions))
    prefix, rest, moved_dma = [], [], 0
    for ins in main.instructions:
        if moved_dma < NSETS and ins.engine == sp and type(ins).__name__ in (
                "InstDMACopy", "InstEventSemaphore"):
            prefix.append(ins)
            if type(ins).__name__ == "InstDMACopy":
                moved_dma += 1
        else:
            rest.append(ins)
    assert moved_dma == NSETS
    main.instructions = rest
    pre.instructions = prefix + pre.instructions
    return nc


def build_nc(legalize=True):
    nc = bass.Bass("TRN2", target_bir_lowering=False, debug=False,
                   num_devices=NCORES)
    blobS = nc.dram_tensor("blobS", [2, P, BPE * 512 // 2], BF16,
                           kind="ExternalInput")
    blobL = nc.dram_tensor("blobL", [3, P, BPE * 1024 // 2], BF16,
                           kind="ExternalInput")
    res_d = nc.dram_tensor("res", [P, 3 * NSETS], F32, kind="ExternalOutput")
    cnt_d = nc.dram_tensor("cnt", [1, 3 * 512], F32, kind="ExternalOutput")

    from contextlib import ExitStack
    with ExitStack() as es:
        block = es.enter_context(nc.Block())
        dsems = [es.enter_context(nc.semaphore(f"ds{i}"))
                 for i in range(NSETS)]
        vs = es.enter_context(nc.semaphore("vs"))    # DVE progress
        as_ = es.enter_context(nc.semaphore("as_"))  # ACT progress
        ps = es.enter_context(nc.semaphore("ps"))    # PE progress
        os_ = es.enter_context(nc.semaphore("os_"))  # output DMA done
        cks = list(CKS)
        tiles = [es.enter_context(
            nc.sbuf_tensor(f"t{i}", [P, BPE * cks[i] // 2], BF16))
            for i in range(NSETS)]
        scratch = [tuple(es.enter_context(
            nc.sbuf_tensor(f"{nm}{i}", [P, cks[i]], BF16))
            for nm in ("d0", "d1", "lg")) for i in range(NSETS)]
        ones = es.enter_context(nc.sbuf_tensor("ones", [P, 1], BF16))
        dummy = es.enter_context(nc.sbuf_tensor("dmy", [P, 1], BF16))
        res_sb = es.enter_context(
            nc.sbuf_tensor("res_sb", [P, 3 * NSETS], F32))
        cnt_sb = es.enter_context(
            nc.sbuf_tensor("cnt_sb", [1, 3 * 512], F32))
        cnt_ps = [es.enter_context(nc.psum_tensor(f"cntp{b}", [1, 512], F32))
                  for b in range(BPC)]
        s2p = es.enter_context(nc.psum_tensor("s2p", [1, 512], F32))

        def fields(s):
            T, ck = tiles[s], cks[s]
            m = T[:, 0:ck]
            conf8 = T[:, ck:2 * ck].bitcast(FP8)
            m_i8 = m.bitcast(mybir.dt.int8).rearrange(
                "p (k two) -> p k two", two=2)[:, :, 1]
            return dict(m=m, m_i8=m_i8, c0=conf8[:, 0:ck],
                        c1=conf8[:, ck:2 * ck],
                        o0=T[:, 2 * ck:3 * ck], o1=T[:, 3 * ck:4 * ck],
                        g0=T[:, 4 * ck:5 * ck], g1=T[:, 5 * ck:6 * ck])

        # semaphore value bookkeeping (emission-order counters)
        V = {"vs": 0, "as": 0, "ps": 0}
        # per-chunk landmark values, filled as streams are emitted
        vget = {}   # ("gather"|"mul0"|"mul1"|"lgm", s) -> vs value
        aget = {}   # ("ln", s) -> as_ value
        pget = {}   # ("cnt", b) / ("s2",) -> ps value

        # ---------------- DVE ----------------
        @block.vector
        def _(vector):
            v = nc.vector
            v.memset(ones[:], 1.0).then_inc(vs, 1)
            V["vs"] += 1
            for s in range(NSETS):
                f = fields(s)
                d0, d1, lg = scratch[s]
                vector.wait_ge(dsems[s], 16)
                v.copy_predicated(f["c0"], f["m_i8"], f["c1"]) \
                    .then_inc(vs, 1)
                V["vs"] += 1
                vget[("gather", s)] = V["vs"]
                v.tensor_sub(d0[:], f["g0"], f["o0"])
                v.tensor_sub(d1[:], f["g1"], f["o1"]).then_inc(vs, 1)
                V["vs"] += 1
                # same-queue RAW (sub -> mul) needs an explicit token for
                # the race model; in-order completion makes one wait enough
                vector.wait_ge(vs, V["vs"])
                v.tensor_mul(d0[:], d0[:], f["m"]).then_inc(vs, 1)
                V["vs"] += 1
                vget[("mul0", s)] = V["vs"]
                v.tensor_mul(d1[:], d1[:], f["m"]).then_inc(vs, 1)
                V["vs"] += 1
                vget[("mul1", s)] = V["vs"]
                # lgm = log(g) * m for the PE S2 reduction
                vector.wait_ge(as_, s + 1)         # Ln of chunk s done
                v.tensor_mul(lg[:], lg[:], f["m"]).then_inc(vs, 1)
                V["vs"] += 1
                vget[("lgm", s)] = V["vs"]
                if BLAST[s]:
                    b = SBATCH[s]
                    vector.wait_ge(ps, b + 1)   # cnt group b stopped
                    v.tensor_copy(cnt_sb[0:1, 512 * b:512 * (b + 1)],
                                  cnt_ps[b][:]).then_inc(vs, 1)
                    V["vs"] += 1
                    vget[("cntcp", b)] = V["vs"]
            # S2 psum -> sbuf (ps==3 once the S2 group stops)
            vector.wait_ge(ps, 3)
            v.tensor_copy(cnt_sb[0:1, 1024:1536], s2p[:]).then_inc(vs, 1)
            V["vs"] += 1
            vget[("s2cp",)] = V["vs"]

        # ---------------- ACT ----------------
        @block.scalar
        def _(scalar):
            sc = nc.scalar
            scalar.wait_ge(vs, 1)              # ones ready
            sc.activation(dummy[:], ones[:], AF.Ln)   # table prefetch
            for s in range(NSETS):
                f = fields(s)
                d0, d1, lg = scratch[s]
                scalar.wait_ge(vs, vget[("gather", s)])
                sc.activation(lg[:], f["c0"], AF.Ln,
                              accum_out=res_sb[:, s:s + 1]).then_inc(as_, 1)
                V["as"] += 1
                aget[("ln", s)] = V["as"]
                scalar.wait_ge(vs, vget[("mul0", s)])
                i = sc.activation(d0[:], d0[:], AF.Square,
                                  accum_out=res_sb[:, NSETS + 2 * s:
                                                   NSETS + 2 * s + 1])
                scalar.wait_ge(vs, vget[("mul1", s)])
                i = sc.activation(d1[:], d1[:], AF.Square,
                                  accum_out=res_sb[:, NSETS + 2 * s + 1:
                                                   NSETS + 2 * s + 2])
                if s == NSETS - 1:
                    i.then_inc(as_, 1)         # final accumulate landmark
                    V["as"] += 1

        # ---------------- PE -----------------
        @block.tensor
        def _(tensor):
            t = nc.tensor
            # per-batch count groups + one global S2 group
            cnt_started = [False, False]
            s2_started = False
            tensor.wait_ge(vs, 1)              # ones ready
            for s in range(NSETS):
                f = fields(s)
                ck = cks[s]
                b = SBATCH[s]
                d0, d1, lg = scratch[s]
                tensor.wait_ge(dsems[s], 16)
                off = 0
                while off < ck:
                    w = min(512, ck - off)
                    last = BLAST[s] and (off + w >= ck)
                    i = t.matmul(cnt_ps[b][:, 0:w], ones[:],
                                 f["m"][:, off:off + w],
                                 start=not cnt_started[b], stop=last,
                                 skip_group_check=True)
                    cnt_started[b] = True
                    off += w
                    if last:
                        i.then_inc(ps, 1)
                        V["ps"] += 1
                        pget[("cnt", b)] = V["ps"]
                tensor.wait_ge(vs, vget[("lgm", s)])
                off = 0
                while off < ck:
                    w = min(512, ck - off)
                    last = (s == NSETS - 1) and (off + w >= ck)
                    i = t.matmul(s2p[:, 0:w], ones[:], lg[:, off:off + w],
                                 start=not s2_started, stop=last,
                                 skip_group_check=True)
                    s2_started = True
                    off += w
                    if last:
                        i.then_inc(ps, 1)
                        V["ps"] += 1
                        pget[("s2",)] = V["ps"]

        # fix the DVE waits that referenced PE progress: recompute final
        # values now that PE stream is emitted (ps: cnt_b0=1 after chunk1,
        # cnt_b1=2 after chunk3, s2=3 at end) -- the wait_ge calls above
        # used these exact constants; assert they match.
        assert pget[("cnt", 0)] == 1 and pget[("cnt", 1)] == 2
        assert pget[("s2",)] == 3 and V["as"] == NSETS + 1

        # ---------------- SP (DMAs) ----------------
        @block.sync
        def _(sync):
            # all four input transfers issued back-to-back; the rings
            # process their descriptors mostly FIFO at ~430 B/ns, and
            # _hoist_input_dmas moves these triggers to the head of the
            # preamble so generation starts as soon as SP is live (~6us).
            # NOTE: never put waits between them -- a blocked SP queue in
            # the preamble holds the all-engine barrier hostage.
            si = li = 0
            for s in range(NSETS):
                blb = (blobS, blobL)[cks[s] != 512]
                idx = si if cks[s] == 512 else li
                if cks[s] == 512:
                    si += 1
                else:
                    li += 1
                nc.sync.dma_start(tiles[s][:], blb[idx]).then_inc(dsems[s],
                                                                 16)
            # cnt's dependency (s2 psum copy) resolves ~1us before the
            # final ACT accumulate -> issue it first
            sync.wait_ge(vs, vget[("s2cp",)])
            nc.sync.dma_start(cnt_d[:, :], cnt_sb[:]).then_inc(os_, 16)
            sync.wait_ge(as_, NSETS + 1)
            nc.sync.dma_start(res_d[:, :], res_sb[:]).then_inc(os_, 16)
            sync.wait_ge(os_, 32)

    if not legalize:
        return nc
    return _legalize_single_wait(_hoist_input_dmas(nc))


_NC = None


def _get_nc():
    global _NC
    if _NC is None:
        _NC = build_nc()
    return _NC


def make_in_maps(confidence, offset, instance, gt_offset):
    conf = np.ascontiguousarray(confidence, dtype=np.float32) \
        .reshape(B, C, P, FREE).astype(NPFP8)
    off = np.ascontiguousarray(offset, dtype=np.float32) \
        .reshape(B, 2, P, FREE).astype(NPBF16)
    gto = np.ascontiguousarray(gt_offset, dtype=np.float32) \
        .reshape(B, 2, P, FREE).astype(NPBF16)
    mask = (np.asarray(instance).reshape(B, P, FREE) != 0).astype(NPBF16)

    def pack(b, lo, hi):
        # byte-level pack: [mask bf16 | c0 fp8 | c1 fp8 | o0 o1 g0 g1 bf16]
        parts = [np.ascontiguousarray(mask[b][:, lo:hi]).view(np.uint8),
                 np.ascontiguousarray(conf[b, 0][:, lo:hi]).view(np.uint8),
                 np.ascontiguousarray(conf[b, 1][:, lo:hi]).view(np.uint8),
                 np.ascontiguousarray(off[b, 0][:, lo:hi]).view(np.uint8),
                 np.ascontiguousarray(off[b, 1][:, lo:hi]).view(np.uint8),
                 np.ascontiguousarray(gto[b, 0][:, lo:hi]).view(np.uint8),
                 np.ascontiguousarray(gto[b, 1][:, lo:hi]).view(np.uint8)]
        return np.concatenate(parts, axis=1).view(NPBF16)

    in_maps = []
    for k in range(NCORES):
        bs = [BPC * k + i for i in range(BPC)]
        packs = [pack(bs[SBATCH[s]], *COLR[s]) for s in range(NSETS)]
        blobS = np.stack([p for s, p in enumerate(packs) if CKS[s] == 512])
        blobL = np.stack([p for s, p in enumerate(packs) if CKS[s] != 512])
        in_maps.append({"blobS": blobS, "blobL": blobL})
    return in_maps


def combine_partials(parts):
    """parts: list of 8 dicts (res [P,12], cnt [1,1536]) -> loss."""
    s1 = sum(p["res"][:, 0:NSETS].sum(dtype=np.float64) for p in parts)
    s2 = sum(p["cnt"][0, 1024:1536].sum(dtype=np.float64) for p in parts)
    n = float(B * H * W)
    conf_loss = -(0.4 * s1 + 0.6 * s2) / n
    off_loss = 0.0
    for p in parts:
        for bi in range(BPC):
            s = sum(p["res"][:, NSETS + 2 * c:NSETS + 2 * c + 2]
                    .sum(dtype=np.float64)
                    for c in range(NSETS) if SBATCH[c] == bi)
            cntb = p["cnt"][0, 512 * bi:512 * (bi + 1)].sum(dtype=np.float64)
            if cntb > 0.5:
                off_loss += s / cntb
    off_loss /= B
    return conf_loss + off_loss


def kernel(confidence, offset, instance, gt_offset):
    nc = _get_nc()
    in_maps = make_in_maps(confidence, offset, instance, gt_offset)
    res = run_bass_kernel_spmd(nc, in_maps, core_ids=list(range(NCORES)))
    parts = [{k: np.asarray(r[k], dtype=np.float64)
              for k in ("res", "cnt")} for r in res.results]
    return np.array(combine_partials(parts), dtype=np.float32)


# revision 31
# speedup vs baseline: 1.0674x; 1.0060x over previous
"""Trainium2 Bass kernel for nn_COLoss_45457933860953 (raw-bass version).

Loss = mean over all pixels of weighted -log(conf gathered by instance)
     + mean over batches of (masked offset MSE sum / fg count).

Data-parallel over the batch dim: 16 batches -> 8 cores x 2 batches.

Host-side compression (loss tolerance 2e-2, measured quantization error
~2e-4): conf channels as fp8-e4m3 (they only feed the gather + Ln; ACT
is dtype-independent), offsets/gt as bf16 (keeps DVE in packed 2x perf
mode), instance mask as bf16 0/1 (arithmetic mask; its high byte is the
int8 predicate copy_predicated needs). Everything for one col-chunk is
one contiguous blob -> one fully-contiguous 2D DMA per chunk:

  per partition, ck cols: [ mask bf16 | c0 fp8 | c1 fp8 | o0 | o1 | g0 | g1 ]

Partition p holds image rows 4p..4p+3 flattened (2048 cols per batch),
col-chunks (1024, 1024) per batch.

RAW bass (no TileContext): the Tile framework costs ~7us of pool-alloc
barriers before the first DMA trigger and ~6us of per-semaphore
postamble teardown (~57 serial waits per engine).  With a hand-built
static schedule we use 10 semaphores total, and a post-pass hoists the
four input-DMA triggers to the head of the preamble block so descriptor
generation starts as soon as the SP queue is live (~6us, vs ~7.3us
after the all-engine barrier).  All four transfers are issued
back-to-back (the rings process them near-FIFO at ~430 B/ns; never put
waits between them -- a blocked SP queue in the preamble holds the
all-engine barrier hostage).

Per chunk s, engines:
  PE  : fg count += ones^T @ mask strips -> psum per batch
        S2 += ones^T @ (m*log g) strips  -> psum (one accumulator)
  DVE : gather g=conf[inst] (copy_predicated on fp8 bytes),
        d_c = gt_c - off_c, d_c *= m, lgm = log(g) * m
  ACT : log g -> bf16 (accum S1 -> psum), Square(d_c*m) (accum -> psum)

Outputs: res[128,12] (S1 per chunk + off sq-sums per chunk*ch),
cnt[1,1536] (= count_b0 | count_b1 | S2 columns). Host combines in
float64.
"""

import sys

if "/opt/trn_rl_repo" not in sys.path:
    sys.path.insert(0, "/opt/trn_rl_repo")

import ml_dtypes
import numpy as np

import concourse.bass as bass
from concourse import mybir
from concourse.bass_utils import run_bass_kernel_spmd

B, C, H, W = 16, 2, 512, 512
NCORES = 8
BPC = B // NCORES            # batches per core
P = 128                      # SBUF partitions
FREE = (H * W) // P          # 2048 cols per partition per image
# chunk tables: batch 0 leads with two 512-col chunks so compute starts
# ~3us earlier (smaller first transfer); the rest are 1024-col chunks.
CKS = (512, 512, 1024, 1024, 1024)       # cols per chunk-set
COLR = ((0, 512), (512, 1024), (1024, 2048), (0, 1024), (1024, 2048))
SBATCH = (0, 0, 0, 1, 1)                 # batch of each chunk-set
BLAST = (False, False, True, False, True)  # last chunk of its batch
NSETS = len(CKS)
BPE = 12                     # blob bytes per col (2+1+1+2+2+2+2)

BF16 = mybir.dt.bfloat16
FP8 = mybir.dt.float8e4
F32 = mybir.dt.float32
AF = mybir.ActivationFunctionType
ALU = mybir.AluOpType
NPBF16 = ml_dtypes.bfloat16
NPFP8 = ml_dtypes.float8_e4m3fn


def _legalize_single_wait(nc):
    """Drop the EVENT_SEMAPHORE_RANGE_CLEAR InstISA this toolchain's walrus
    rejects, and hoist surplus sync-waits (>1) onto standalone single-wait
    carriers on the same queue (prefix waits on an in-order queue are
    semantically identical to instruction waits)."""
    cnt = 0
    for f in nc.m.functions:
        for blk in f.blocks:
            out = []
            for ins in blk.instructions:
                nm = type(ins).__name__
                if (nm == "InstISA" and
                        getattr(ins, "op_name", None) ==
                        "EVENT_SEMAPHORE_RANGE_CLEAR"):
                    continue
                si = getattr(ins, "sync_info", None)
                if si is not None and si.on_wait and len(si.on_wait) > 1:
                    waits = list(si.on_wait)
                    for w in waits[:-1]:
                        cnt += 1
                        out.append(mybir.InstEventSemaphore(
                            name=f"{ins.name}-hoist{cnt}",
                            engine=ins.engine,
                            ins=[], outs=[],
                            sync_info=mybir.SyncInfo(on_wait=[w],
                                                     on_update=[]),
                        ))
                    ins.sync_info = mybir.SyncInfo(
                        on_wait=[waits[-1]], on_update=list(si.on_update))
                out.append(ins)
            blk.instructions = out
    return nc


def _hoist_input_dmas(nc):
    """Move the SP-engine input-DMA prefix (4 DMACopy triggers + their
    serialize waits) from the main block to the FRONT of the preamble
    block.  The preamble's all-engine barrier otherwise delays the first
    descriptor generation to ~7.3us; hoisted, the SP queue generates
    descriptors as soon as it is live (~4.6us) while the other engines
    run their preamble.  Only the SP-engine subsequence order matters,
    and the data dependencies (dsem waits by compute) are unchanged."""
    blocks = nc.m.functions[0].blocks
    pre = blocks[0]
    sp = mybir.EngineType.SP
    main = next(b for b in blocks[1:]
                if any(type(i).__name__ == "InstDMACopy" and i.engine == sp
                       for i in b.instructions))
    prefix, rest, moved_dma = [], [], 0
    for ins in main.instructions:
        if moved_dma < NSETS and ins.engine == sp and type(ins).__name__ in (
                "InstDMACopy", "InstEventSemaphore"):
            prefix.append(ins)
            if type(ins).__name__ == "InstDMACopy":
                moved_dma += 1
        else:
            rest.append(ins)
    assert moved_dma == NSETS
    main.instructions = rest
    pre.instructions = prefix + pre.instructions
    return nc


def build_nc(legalize=True):
    nc = bass.Bass("TRN2", target_bir_lowering=False, debug=False,
                   num_devices=NCORES)
    blobS = nc.dram_tensor("blobS", [2, P, BPE * 512 // 2], BF16,
                           kind="ExternalInput")
    blobL = nc.dram_tensor("blobL", [3, P, BPE * 1024 // 2], BF16,
                           kind="ExternalInput")
    res_d = nc.dram_tensor("res", [P, 3 * NSETS], F32, kind="ExternalOutput")
    cnt_d = nc.dram_tensor("cnt", [1, 3 * 512], F32, kind="ExternalOutput")

    from contextlib import ExitStack
    with ExitStack() as es:
        block = es.enter_context(nc.Block())
        dsems = [es.enter_context(nc.semaphore(f"ds{i}"))
                 for i in range(NSETS)]
        vs = es.enter_context(nc.semaphore("vs"))    # DVE progress
        as_ = es.enter_context(nc.semaphore("as_"))  # ACT progress
        ps = es.enter_context(nc.semaphore("ps"))    # PE progress
        os_ = es.enter_context(nc.semaphore("os_"))  # output DMA done
        cks = list(CKS)
        tiles = [es.enter_context(
            nc.sbuf_tensor(f"t{i}", [P, BPE * cks[i] // 2], BF16))
            for i in range(NSETS)]
        scratch = [tuple(es.enter_context(
            nc.sbuf_tensor(f"{nm}{i}", [P, cks[i]], BF16))
            for nm in ("d0", "d1", "lg")) for i in range(NSETS)]
        ones = es.enter_context(nc.sbuf_tensor("ones", [P, 1], BF16))
        dummy = es.enter_context(nc.sbuf_tensor("dmy", [P, 1], BF16))
        res_sb = es.enter_context(
            nc.sbuf_tensor("res_sb", [P, 3 * NSETS], F32))
        cnt_sb = es.enter_context(
            nc.sbuf_tensor("cnt_sb", [1, 3 * 512], F32))
        cnt_ps = [es.enter_context(nc.psum_tensor(f"cntp{b}", [1, 512], F32))
                  for b in range(BPC)]
        s2p = es.enter_context(nc.psum_tensor("s2p", [1, 512], F32))

        def fields(s):
            T, ck = tiles[s], cks[s]
            m = T[:, 0:ck]
            conf8 = T[:, ck:2 * ck].bitcast(FP8)
            m_i8 = m.bitcast(mybir.dt.int8).rearrange(
                "p (k two) -> p k two", two=2)[:, :, 1]
            return dict(m=m, m_i8=m_i8, c0=conf8[:, 0:ck],
                        c1=conf8[:, ck:2 * ck],
                        o0=T[:, 2 * ck:3 * ck], o1=T[:, 3 * ck:4 * ck],
                        g0=T[:, 4 * ck:5 * ck], g1=T[:, 5 * ck:6 * ck])

        # semaphore value bookkeeping (emission-order counters)
        V = {"vs": 0, "as": 0, "ps": 0}
        # per-chunk landmark values, filled as streams are emitted
        vget = {}   # ("gather"|"mul0"|"mul1"|"lgm", s) -> vs value
        aget = {}   # ("ln", s) -> as_ value
        pget = {}   # ("cnt", b) / ("s2",) -> ps value

        # ---------------- DVE ----------------
        @block.vector
        def _(vector):
            v = nc.vector
            v.memset(ones[:], 1.0).then_inc(vs, 1)
            V["vs"] += 1
            for s in range(NSETS):
                f = fields(s)
                d0, d1, lg = scratch[s]
                vector.wait_ge(dsems[s], 16)
                v.copy_predicated(f["c0"], f["m_i8"], f["c1"]) \
                    .then_inc(vs, 1)
                V["vs"] += 1
                vget[("gather", s)] = V["vs"]
                v.tensor_sub(d0[:], f["g0"], f["o0"])
                v.tensor_sub(d1[:], f["g1"], f["o1"]).then_inc(vs, 1)
                V["vs"] += 1
                # same-queue RAW (sub -> mul) needs an explicit token for
                # the race model; in-order completion makes one wait enough
                vector.wait_ge(vs, V["vs"])
                v.tensor_mul(d0[:], d0[:], f["m"]).then_inc(vs, 1)
                V["vs"] += 1
                vget[("mul0", s)] = V["vs"]
                v.tensor_mul(d1[:], d1[:], f["m"]).then_inc(vs, 1)
                V["vs"] += 1
                vget[("mul1", s)] = V["vs"]
                # lgm = log(g) * m for the PE S2 reduction
                vector.wait_ge(as_, s + 1)         # Ln of chunk s done
                v.tensor_mul(lg[:], lg[:], f["m"]).then_inc(vs, 1)
                V["vs"] += 1
                vget[("lgm", s)] = V["vs"]
                if BLAST[s]:
                    b = SBATCH[s]
                    vector.wait_ge(ps, b + 1)   # cnt group b stopped
                    v.tensor_copy(cnt_sb[0:1, 512 * b:512 * (b + 1)],
                                  cnt_ps[b][:]).then_inc(vs, 1)
                    V["vs"] += 1
                    vget[("cntcp", b)] = V["vs"]
            # S2 psum -> sbuf (ps==3 once the S2 group stops)
            vector.wait_ge(ps, 3)
            v.tensor_copy(cnt_sb[0:1, 1024:1536], s2p[:]).then_inc(vs, 1)
            V["vs"] += 1
            vget[("s2cp",)] = V["vs"]

        # ---------------- ACT ----------------
        @block.scalar
        def _(scalar):
            sc = nc.scalar
            scalar.wait_ge(vs, 1)              # ones ready
            sc.activation(dummy[:], ones[:], AF.Ln)   # table prefetch
            for s in range(NSETS):
                f = fields(s)
                d0, d1, lg = scratch[s]
                scalar.wait_ge(vs, vget[("gather", s)])
                sc.activation(lg[:], f["c0"], AF.Ln,
                              accum_out=res_sb[:, s:s + 1]).then_inc(as_, 1)
                V["as"] += 1
                aget[("ln", s)] = V["as"]
                scalar.wait_ge(vs, vget[("mul0", s)])
                i = sc.activation(d0[:], d0[:], AF.Square,
                                  accum_out=res_sb[:, NSETS + 2 * s:
                                                   NSETS + 2 * s + 1])
                scalar.wait_ge(vs, vget[("mul1", s)])
                i = sc.activation(d1[:], d1[:], AF.Square,
                                  accum_out=res_sb[:, NSETS + 2 * s + 1:
                                                   NSETS + 2 * s + 2])
                if s == NSETS - 1:
                    i.then_inc(as_, 1)         # final accumulate landmark
                    V["as"] += 1
            # cnt output DMA triggered from the ACT queue (idle by now) so
            # its descriptor gen runs in parallel with res's on SP
            scalar.wait_ge(vs, vget[("s2cp",)])
            nc.scalar.dma_start(cnt_d[:, :], cnt_sb[:]).then_inc(os_, 16)

        # ---------------- PE -----------------
        @block.tensor
        def _(tensor):
            t = nc.tensor
            # per-batch count groups + one global S2 group
            cnt_started = [False, False]
            s2_started = False
            tensor.wait_ge(vs, 1)              # ones ready
            for s in range(NSETS):
                f = fields(s)
                ck = cks[s]
                b = SBATCH[s]
                d0, d1, lg = scratch[s]
                tensor.wait_ge(dsems[s], 16)
                off = 0
                while off < ck:
                    w = min(512, ck - off)
                    last = BLAST[s] and (off + w >= ck)
                    i = t.matmul(cnt_ps[b][:, 0:w], ones[:],
                                 f["m"][:, off:off + w],
                                 start=not cnt_started[b], stop=last,
                                 skip_group_check=True)
                    cnt_started[b] = True
                    off += w
                    if last:
                        i.then_inc(ps, 1)
                        V["ps"] += 1
                        pget[("cnt", b)] = V["ps"]
                tensor.wait_ge(vs, vget[("lgm", s)])
                off = 0
                while off < ck:
                    w = min(512, ck - off)
                    last = (s == NSETS - 1) and (off + w >= ck)
                    i = t.matmul(s2p[:, 0:w], ones[:], lg[:, off:off + w],
                                 start=not s2_started, stop=last,
                                 skip_group_check=True)
                    s2_started = True
                    off += w
                    if last:
                        i.then_inc(ps, 1)
                        V["ps"] += 1
                        pget[("s2",)] = V["ps"]

        # fix the DVE waits that referenced PE progress: recompute final
        # values now that PE stream is emitted (ps: cnt_b0=1 after chunk1,
        # cnt_b1=2 after chunk3, s2=3 at end) -- the wait_ge calls above
        # used these exact constants; assert they match.
        assert pget[("cnt", 0)] == 1 and pget[("cnt", 1)] == 2
        assert pget[("s2",)] == 3 and V["as"] == NSETS + 1

        # ---------------- SP (DMAs) ----------------
        @block.sync
        def _(sync):
            # all four input transfers issued back-to-back; the rings
            # process their descriptors mostly FIFO at ~430 B/ns, and
            # _hoist_input_dmas moves these triggers to the head of the
            # preamble so generation starts as soon as SP is live (~6us).
            # NOTE: never put waits between them -- a blocked SP queue in
            # the preamble holds the all-engine barrier hostage.
            si = li = 0
            for s in range(NSETS):
                blb = (blobS, blobL)[cks[s] != 512]
                idx = si if cks[s] == 512 else li
                if cks[s] == 512:
                    si += 1
                else:
                    li += 1
                nc.sync.dma_start(tiles[s][:], blb[idx]).then_inc(dsems[s],
                                                                 16)
            # res's dependency (final ACT accumulate, ~31.9us) resolves
            # before cnt's (s2 psum copy, ~32.3us): issue res here while
            # the PE queue handles cnt in parallel
            sync.wait_ge(as_, NSETS + 1)
            nc.sync.dma_start(res_d[:, :], res_sb[:]).then_inc(os_, 16)
            sync.wait_ge(os_, 32)

    if not legalize:
        return nc
    return _legalize_single_wait(_hoist_input_dmas(nc))


_NC = None


def _get_nc():
    global _NC
    if _NC is None:
        _NC = build_nc()
    return _NC


def make_in_maps(confidence, offset, instance, gt_offset):
    conf = np.ascontiguousarray(confidence, dtype=np.float32) \
        .reshape(B, C, P, FREE).astype(NPFP8)
    off = np.ascontiguousarray(offset, dtype=np.float32) \
        .reshape(B, 2, P, FREE).astype(NPBF16)
    gto = np.ascontiguousarray(gt_offset, dtype=np.float32) \
        .reshape(B, 2, P, FREE).astype(NPBF16)
    mask = (np.asarray(instance).reshape(B, P, FREE) != 0).astype(NPBF16)

    def pack(b, lo, hi):
        # byte-level pack: [mask bf16 | c0 fp8 | c1 fp8 | o0 o1 g0 g1 bf16]
        parts = [np.ascontiguousarray(mask[b][:, lo:hi]).view(np.uint8),
                 np.ascontiguousarray(conf[b, 0][:, lo:hi]).view(np.uint8),
                 np.ascontiguousarray(conf[b, 1][:, lo:hi]).view(np.uint8),
                 np.ascontiguousarray(off[b, 0][:, lo:hi]).view(np.uint8),
                 np.ascontiguousarray(off[b, 1][:, lo:hi]).view(np.uint8),
                 np.ascontiguousarray(gto[b, 0][:, lo:hi]).view(np.uint8),
                 np.ascontiguousarray(gto[b, 1][:, lo:hi]).view(np.uint8)]
        return np.concatenate(parts, axis=1).view(NPBF16)

    in_maps = []
    for k in range(NCORES):
        bs = [BPC * k + i for i in range(BPC)]
        packs = [pack(bs[SBATCH[s]], *COLR[s]) for s in range(NSETS)]
        blobS = np.stack([p for s, p in enumerate(packs) if CKS[s] == 512])
        blobL = np.stack([p for s, p in enumerate(packs) if CKS[s] != 512])
        in_maps.append({"blobS": blobS, "blobL": blobL})
    return in_maps


def combine_partials(parts):
    """parts: list of 8 dicts (res [P,12], cnt [1,1536]) -> loss."""
    s1 = sum(p["res"][:, 0:NSETS].sum(dtype=np.float64) for p in parts)
    s2 = sum(p["cnt"][0, 1024:1536].sum(dtype=np.float64) for p in parts)
    n = float(B * H * W)
    conf_loss = -(0.4 * s1 + 0.6 * s2) / n
    off_loss = 0.0
    for p in parts:
        for bi in range(BPC):
            s = sum(p["res"][:, NSETS + 2 * c:NSETS + 2 * c + 2]
                    .sum(dtype=np.float64)
                    for c in range(NSETS) if SBATCH[c] == bi)
            cntb = p["cnt"][0, 512 * bi:512 * (bi + 1)].sum(dtype=np.float64)
            if cntb > 0.5:
                off_loss += s / cntb
    off_loss /= B
    return conf_loss + off_loss


def kernel(confidence, offset, instance, gt_offset):
    nc = _get_nc()
    in_maps = make_in_maps(confidence, offset, instance, gt_offset)
    res = run_bass_kernel_spmd(nc, in_maps, core_ids=list(range(NCORES)))
    parts = [{k: np.asarray(r[k], dtype=np.float64)
              for k in ("res", "cnt")} for r in res.results]
    return np.array(combine_partials(parts), dtype=np.float32)
